# revision 21
# baseline (speedup 1.0000x reference)
"""Trainium2 Bass kernel for nn_AnticipatoryTransformer (8 NeuronCores).

v2 strategy (sequence-parallel, self-contained):
  - 2048 tokens (B=2 x S=1024) sharded 8 ways: core c handles batch b=c//4,
    rank p=c%4 of a 4-core group; 32-row striping (256 tokens/core).
  - Per layer: LN1 -> y^T (bf16 matmul transposes) -> K^T first, AllGather-K
    launched early; V next, AllGather-V; Q last (overlaps the collectives).
  - Attention: per head-pair (64-row PE groups run concurrently), per
    k~-tile-pair: scores into a tau-merged [128, Wp] PSUM bank, one merged
    fp8 bias matmul (trajectory+causal+window, host-packed, masked lanes get
    -224 => exp -> 0), one exp per head over the merged tile, AV accumulate
    with an extra ones column yielding softmax denominators.  Normalize via
    reciprocal_approx_fast + gpsimd partition_broadcast + DVE multiply.
  - QKV/out-proj weights + attention bias in fp8e4m3 (dequant scale folded
    into the PSUM->SBUF copies / residual adds), FFN weights bf16.
  - Weight loads as ~0.5-2MB slab DMAs; PSUM managed as one shared 8-bank
    pool; elementwise PSUM drains on DVE; gelu batched [128,512].
"""

import numpy as np
import ml_dtypes

BF16 = ml_dtypes.bfloat16
F8 = ml_dtypes.float8_e4m3
B, S, D, H, DH, L, FF, W = 2, 1024, 1024, 16, 64, 4, 4096, 256
NEG = -1e9
MASK8 = -224.0          # fp8 bias mask; exp(score + MASK8) == 0 in fp32
EPS = 1e-5
GROUP = 4
NCORE = 8
TPC = 256          # tokens per core
VE = 65            # V columns per head incl. ones column
VEXT = H * VE      # 1040

LAST_RESULT = None


def _gtok(rank, t):
    return 32 * (4 * (t // 32) + rank) + t % 32


LOCAL2GLOBAL = {p: np.array([_gtok(p, j) for j in range(TPC)]) for p in range(4)}
KTILDE2GLOBAL = np.array([_gtok(r, t) for r in range(4) for t in range(TPC)])


def _colrange(parity, tau):
    """Active q~ column range for a k~-tile with t-half tau, given layer parity."""
    if parity == 1:  # odd layer: causal only
        return (128 * tau, 256)
    return (max(0, 32 * (4 * tau - 1)), min(256, 32 * (4 * tau + 5)))


def _pair_geom(parity):
    """(c00,c01,c10,c11,W0,Wp) for a tau-merged k~-tile pair."""
    c00, c01 = _colrange(parity, 0)
    c10, c11 = _colrange(parity, 1)
    W0 = c01 - c00
    Wp = W0 + (c11 - c10)
    return c00, c01, c10, c11, W0, Wp


def build_nc(bass, tile, mybir, n_layers=L, qkvb_nz=False, v_bias_nz=False,
             b1_nz=False, b2_nz=False, gate_consts=(0.0, 1.0, 1.0, 0.0),
             inv_s_qkv=1.0, inv_s_out=1.0):
    """Build the SPMD Bass graph (identical on all 8 cores)."""
    gate_b_c, gc0_c, gc1_c, gcb_c = (float(v) for v in gate_consts)
    from contextlib import ExitStack

    dt = mybir.dt
    AF = mybir.ActivationFunctionType
    OP = mybir.AluOpType

    nc = bass.Bass("TRN2", target_bir_lowering=False, debug=False,
                   num_devices=NCORE)

    f32, bf16, f8 = dt.float32, dt.bfloat16, dt.float8e4
    din = lambda name, shape, d: nc.dram_tensor(name, shape, d, kind="ExternalInput")

    WP = [_pair_geom(0)[5], _pair_geom(1)[5]]      # 320, 384
    S_K = 8.0   # K stored as fp8 * S_K; descaled via the Q copy
    S_V = 8.0   # V stored as fp8 * S_V; descaled in the epilogue multiply

    x_in = din("x_sh", [TPC, D], f32)
    biasp_e = din("biasp_e", [8, 128, 8 * WP[0]], f8)
    biasp_o = din("biasp_o", [8, 128, 8 * WP[1]], f8)
    qkvw = din("qkvw", [n_layers, 6, 128, 8 * 512], f8)     # [l,fg,p,(ci c)]
    outw = din("outw", [n_layers, 128, 8 * 1024], f8)       # [l,p,(di cc c)]
    w1p = din("w1p", [n_layers, 8, 128, 8 * 512], bf16)     # [l,ffg,p,(ci c)]
    w2p = din("w2p", [n_layers, 8, 128, 4 * 1024], bf16)    # [l,g,p,(f4 c)]
    hw1p = din("hw1p", [128, 8 * 512], bf16)                # [p,(ci c)]
    hw2p = din("hw2p", [D // 2, 7], bf16)
    gwp = din("gwp", [128, D], f32)
    identf = din("identf", [128, 128], f32)
    identb = din("identb", [128, 128], bf16)
    ident8 = din("ident8", [128, 128], f8)
    qkvb_p = din("qkvb_p", [n_layers, 16, 128], f32)     # per-partition bias (xS)
    b1e_p = din("b1e_p", [n_layers, 32, 128], f32)
    hb1_p = din("hb1_p", [4, 128], f32)
    hb2_p = din("hb2_p", [7, 1], f32)
    vbl_p = din("vbl_p", [n_layers, 1, D], bf16)
    b2l_p = din("b2l_p", [n_layers, 1, D], bf16)

    out_p = nc.dram_tensor("out", [TPC, D + 8], f32, kind="ExternalOutput")

    KSZ = 8 * 128 * TPC
    VSZ = 2 * 128 * VEXT
    cc_kin = nc.dram_tensor("cc_kin", [KSZ], f8)
    cc_kout = nc.dram_tensor("cc_kout", [GROUP * KSZ], f8)
    cc_vin = nc.dram_tensor("cc_vin", [VSZ], f8)
    cc_vout = nc.dram_tensor("cc_vout", [GROUP * VSZ], f8)
    cc_win = nc.dram_tensor("cc_win", [16384], f8)
    cc_wout = nc.dram_tensor("cc_wout", [GROUP * 16384], f8)
    rgroups = [[0, 1, 2, 3], [4, 5, 6, 7]]

    with tile.TileContext(nc) as tc:
        with ExitStack() as ctx:
            pool = lambda name, bufs: ctx.enter_context(tc.tile_pool(name=name, bufs=bufs))
            p_const = pool("const", 1)
            p_h = pool("h", 1)
            p_ybf = pool("ybf", 2)
            p_yt = pool("yt", 1)
            p_qt = pool("qt", 1)
            p_ktl = pool("ktl", 1)
            p_vx = pool("vx", 1)
            p_ktf = pool("ktf", 1)
            p_vf = pool("vf", 1)
            p_h1 = pool("h1", 1)
            p_ot = pool("ot", 1)
            p_wq = pool("wq", 3)
            p_wo = pool("wo", 2)
            p_w1 = pool("w1", 4)
            p_w2 = pool("w2", 3)
            p_eb = pool("eb", 2)
            p_at = pool("at", 6)
            p_small = pool("small", 2)
            p_scr = pool("scr", 2)
            p_stat = pool("stat", 8)
            p_outsb = pool("outsb", 1)
            p_ps = ctx.enter_context(tc.tile_pool(name="ps", bufs=8, space="PSUM"))

            def pst():
                return p_ps.tile([128, 512], f32, tag="ps", name="ps")

            # ---- persistent tiles
            h_sb = [p_h.tile([128, D], f32, tag=f"h{i}", name=f"h{i}") for i in range(2)]
            y_t = [p_yt.tile([128, TPC], bf16, tag=f"yt{i}", name=f"yt{i}") for i in range(8)]
            qt_sb = p_qt.tile([128, 8 * TPC], bf16, tag="qt", name="qt")
            ktl_sb = p_ktl.tile([128, 8 * TPC], f8, tag="ktl", name="ktl")
            vx_l = [p_vx.tile([128, VEXT], f8, tag=f"vx{i}", name=f"vx{i}") for i in range(2)]
            kt_f = [p_ktf.tile([128, 4 * TPC], f8, tag=f"ktf{i}", name=f"ktf{i}") for i in range(8)]
            v_f = [p_vf.tile([128, VEXT], f8, tag=f"vf{i}", name=f"vf{i}") for i in range(8)]
            h1_t = [p_h1.tile([128, 4 * TPC], bf16, tag=f"h1{i}", name=f"h1{i}") for i in range(8)]
            ot_sb = [p_ot.tile([128, TPC], bf16, tag=f"ot{i}", name=f"ot{i}") for i in range(8)]
            idf = p_const.tile([128, 128], f32, tag="idf", name="idf")
            idb = p_const.tile([128, 128], bf16, tag="idb", name="idb")
            id8 = p_const.tile([128, 128], f8, tag="id8", name="id8")
            ones1 = p_const.tile([1, 128], bf16, tag="ones1", name="ones1")
            ones1f = p_const.tile([1, 128], f32, tag="ones1f", name="ones1f")
            gw_b = p_const.tile([128, D], f32, tag="gwb", name="gwb")
            hb2_t = p_const.tile([7, 1], f32, tag="hb2", name="hb2")
            eps_t = p_const.tile([128, 1], f32, tag="epst", name="epst")
            gb_t = p_const.tile([128, 1], f32, tag="gbt", name="gbt")
            gcb_t = p_const.tile([128, 1], f32, tag="gcbt", name="gcbt")
            out_sb = [p_outsb.tile([128, D + 8], f32, tag=f"osb{i}", name=f"osb{i}") for i in range(2)]

            # ---- init
            nc.sync.dma_start(idf[:], identf.ap()[:, :])
            nc.sync.dma_start(idb[:], identb.ap()[:, :])
            nc.sync.dma_start(id8[:], ident8.ap()[:, :])
            nc.sync.dma_start(gw_b[:], gwp.ap()[:, :])
            nc.sync.dma_start(hb2_t[:], hb2_p.ap()[:, :])
            nc.vector.memset(ones1[:], 1.0)
            nc.vector.memset(ones1f[:], 1.0)
            nc.vector.memset(eps_t[:], EPS)
            nc.vector.memset(gb_t[:], gate_b_c)
            nc.vector.memset(gcb_t[:], gcb_c)
            nc.sync.dma_start(
                cc_win.ap().rearrange("(p t) -> p t", p=128), id8[:])
            nc.gpsimd.collective_compute(
                "AllGather", mybir.AluOpType.bypass, replica_groups=rgroups,
                ins=[cc_win.ap().opt()], outs=[cc_wout.ap().opt()])
            for ti in range(2):
                nc.sync.dma_start(h_sb[ti][:], x_in.ap()[ti * 128:(ti + 1) * 128, :])
                ones_ap = vx_l[ti].rearrange("p (h e) -> p h e", e=VE)[:, :, 64:65]
                nc.gpsimd.memset(ones_ap, 1.0)

            def layer_norm_t(dst8):
                """LN of h_sb -> bf16 y (2x[128,D]) -> y^T into dst8 (8x[128,256])."""
                y_bf = [p_ybf.tile([128, D], bf16, tag=f"ybf{i}", name=f"ybf{i}")
                        for i in range(2)]
                for ti in range(2):
                    sq = p_scr.tile([128, D], f32, tag="scr", name="sq")[:]
                    ssum = p_stat.tile([128, 1], f32, tag="ssum", name="ssum")
                    sumsq = p_stat.tile([128, 1], f32, tag="sumsq", name="sumsq")
                    mean = p_stat.tile([128, 1], f32, tag="mean", name="mean")
                    ex2 = p_stat.tile([128, 1], f32, tag="ex2", name="ex2")
                    msq = p_stat.tile([128, 1], f32, tag="msq", name="msq")
                    var = p_stat.tile([128, 1], f32, tag="var", name="var")
                    std = p_stat.tile([128, 1], f32, tag="std", name="std")
                    istd = p_stat.tile([128, 1], f32, tag="istd", name="istd")
                    nc.scalar.activation(sq, h_sb[ti][:], AF.Square,
                                         accum_out=sumsq[:])
                    nc.vector.reduce_sum(ssum[:], h_sb[ti][:], axis=mybir.AxisListType.X)
                    nc.vector.tensor_scalar(mean[:], ssum[:], 1.0 / D, None, OP.mult)
                    nc.vector.tensor_scalar(ex2[:], sumsq[:], 1.0 / D, None, OP.mult)
                    nc.vector.tensor_tensor(msq[:], mean[:], mean[:], OP.mult)
                    nc.vector.tensor_tensor(var[:], ex2[:], msq[:], OP.subtract)
                    nc.scalar.activation(std[:], var[:], AF.Sqrt, bias=eps_t[:])
                    nc.vector.reciprocal(istd[:], std[:])
                    nc.vector.tensor_scalar(y_bf[ti][:], h_sb[ti][:],
                                            mean[:], istd[:], OP.subtract, OP.mult)
                for ci in range(8):
                    ps = pst()
                    for ti in range(2):
                        nc.tensor.matmul(ps[:, ti * 128:(ti + 1) * 128],
                                         y_bf[ti][:, ci * 128:(ci + 1) * 128],
                                         idb[:], start=(ti == 0), stop=(ti == 1),
                                         skip_group_check=True)
                    nc.vector.tensor_copy(dst8[ci][:], ps[:, 0:TPC])

            for l in range(n_layers):
                parity = l % 2
                bias_dram = biasp_o if parity else biasp_e
                c00, c01, c10, c11, W0, Wp = _pair_geom(parity)

                # ======== LN1 + y1^T
                layer_norm_t(y_t)

                if qkvb_nz:
                    qkvb_sb = p_small.tile([128, 16], f32, tag="qkvb", name="qkvb")
                    nc.sync.dma_start(
                        qkvb_sb[:], qkvb_p.ap()[l].rearrange("a b -> b a"))

                # ======== QKV in order K, V, Q with early collectives
                def qk_block(fg, dst, dst_off, scl):
                    wt = p_wq.tile([128, 8 * 512], f8, tag="wq", name="wq")
                    nc.sync.dma_start(wt[:], qkvw.ap()[l, fg])
                    pss = [pst() for _ in range(2)]
                    for ci in range(8):
                        for sub in range(4):
                            nc.tensor.matmul(
                                pss[sub // 2][:, (sub % 2) * 256:(sub % 2) * 256 + 256],
                                wt[:, ci * 512 + sub * 128: ci * 512 + sub * 128 + 128],
                                y_t[ci][:],
                                start=(ci == 0 and sub % 2 == 0),
                                stop=(ci == 7 and sub % 2 == 1),
                                skip_group_check=True)
                    for sub in range(4):
                        fi = fg * 4 + sub - dst_off
                        if qkvb_nz:
                            dsl = dst[:, fi * 256:(fi + 1) * 256]
                            srcp = pss[sub // 2][:, (sub % 2) * 256:(sub % 2) * 256 + 256]
                            nc.vector.tensor_scalar(
                                dsl, srcp, qkvb_sb[:, (fg * 4 + sub):(fg * 4 + sub) + 1],
                                scl, OP.add, OP.mult)
                        elif sub % 2 == 0:
                            fi0 = fg * 4 + sub - dst_off
                            dsl = dst[:, fi0 * 256:(fi0 + 2) * 256]
                            nc.vector.tensor_scalar(dsl, pss[sub // 2][:, 0:512],
                                                    scl, None, OP.mult)

                # K (features 1024:2048 -> fg 2,3)
                for fg in (2, 3):
                    qk_block(fg, ktl_sb, 8, inv_s_qkv * S_K)
                nc.sync.dma_start(
                    cc_kin.ap().rearrange("(f p t) -> p f t", p=128, t=TPC),
                    ktl_sb.rearrange("p (f t) -> p f t", t=TPC))
                nc.gpsimd.collective_compute(
                    "AllGather", mybir.AluOpType.bypass, replica_groups=rgroups,
                    ins=[cc_kin.ap().opt()], outs=[cc_kout.ap().opt()])
                cko = cc_kout.ap().rearrange("(r f p t) -> f p r t", r=GROUP, f=8,
                                             p=128)
                for fi in range(8):
                    nc.scalar.dma_start(kt_f[fi][:], cko[fi])

                # Q (fg 0,1)
                for fg in (0, 1):
                    qk_block(fg, qt_sb, 0, inv_s_qkv / S_K)

                # V (fg 4,5)
                for fg in (4, 5):
                    pss = [pst() for _ in range(2)]
                    wt = p_wq.tile([128, 8 * 512], f8, tag="wq", name="wq")
                    nc.sync.dma_start(wt[:], qkvw.ap()[l, fg])
                    for ci in range(8):
                        for ti in range(2):
                            nc.tensor.matmul(
                                pss[ti][:], y_t[ci][:, ti * 128:(ti + 1) * 128],
                                wt[:, ci * 512:(ci + 1) * 512],
                                start=(ci == 0), stop=(ci == 7),
                                skip_group_check=True)
                    if v_bias_nz:
                        vb_sb = p_small.tile([1, 512], bf16, tag="vb", name="vb")
                        nc.sync.dma_start(
                            vb_sb[:], vbl_p.ap()[l][:, (fg - 4) * 512:(fg - 3) * 512])
                        for ti in range(2):
                            nc.tensor.matmul(pss[ti][:], ones1[:], vb_sb[:],
                                             start=False, stop=True,
                                             skip_group_check=True)
                    h0 = (fg - 4) * 8
                    for ti in range(2):
                        dstv = vx_l[ti].rearrange("p (h e) -> p h e", e=VE)[
                            :, h0:h0 + 8, 0:64]
                        nc.scalar.activation(
                            dstv, pss[ti].rearrange("p (h e) -> p h e", e=64),
                            AF.Copy, scale=inv_s_qkv * S_V)
                for ti in range(2):
                    nc.sync.dma_start(
                        cc_vin.ap().rearrange("(a p f) -> a p f", a=2, p=128)[ti],
                        vx_l[ti][:])
                nc.gpsimd.collective_compute(
                    "AllGather", mybir.AluOpType.bypass, replica_groups=rgroups,
                    ins=[cc_vin.ap().opt()], outs=[cc_vout.ap().opt()])
                cvo = cc_vout.ap().rearrange("(r a p f) -> r a p f", r=GROUP, a=2,
                                             p=128)
                for jt in range(8):
                    nc.scalar.dma_start(v_f[jt][:], cvo[jt // 2, jt % 2])

                # ======== attention
                for fi in range(8):
                    eb = p_eb.tile([128, 8 * Wp], f8, tag="eb", name="eb")
                    nc.sync.dma_start(eb[:], bias_dram.ap()[fi])
                    ps_o = [pst(), pst()]
                    for p4 in range(4):
                        jt0, jt1 = 2 * p4, 2 * p4 + 1
                        ps_s = [pst(), pst()]
                        for hs in range(2):
                            poff = hs * 64
                            nc.tensor.matmul(
                                ps_s[hs][:, 0:W0],
                                kt_f[fi][poff:poff + 64, jt0 * 128:jt0 * 128 + 128],
                                qt_sb[poff:poff + 64, fi * 256 + c00:fi * 256 + c01],
                                start=True, stop=False, skip_group_check=True)
                        for hs in range(2):
                            poff = hs * 64
                            nc.tensor.matmul(
                                ps_s[hs][:, W0:Wp],
                                kt_f[fi][poff:poff + 64, jt1 * 128:jt1 * 128 + 128],
                                qt_sb[poff:poff + 64, fi * 256 + c10:fi * 256 + c11],
                                start=False, stop=False, skip_group_check=True)
                        for hs in range(2):
                            nc.tensor.matmul(
                                ps_s[hs][:, 0:Wp], id8[:],
                                eb[:, (p4 * 2 + hs) * Wp:(p4 * 2 + hs + 1) * Wp],
                                start=False, stop=True, skip_group_check=True)
                        for hs in range(2):
                            at = p_at.tile([128, 384], bf16, tag="at",
                                           name="at")
                            nc.scalar.activation(at[:, 0:Wp], ps_s[hs][:, 0:Wp],
                                                 AF.Exp)
                            hd = 2 * fi + hs
                            nc.tensor.matmul(
                                ps_o[hs][0:VE, c00:c01],
                                v_f[jt0][:, hd * VE:(hd + 1) * VE],
                                at[:, 0:W0],
                                start=(p4 == 0), stop=False, skip_group_check=True)
                            nc.tensor.matmul(
                                ps_o[hs][0:VE, c10:c11],
                                v_f[jt1][:, hd * VE:(hd + 1) * VE],
                                at[:, W0:Wp],
                                start=False, stop=(p4 == 3), skip_group_check=True)
                    for hs in range(2):
                        poff = hs * 64
                        dn = p_small.tile([1, TPC], f32, tag="dn", name="dn")
                        nc.vector.reciprocal(dn[:], ps_o[hs][64:65, 0:TPC])
                        rb_ps = pst()
                        nc.tensor.matmul(rb_ps[0:64, 0:TPC], ones1f[0:1, 0:64],
                                         dn[:], start=True, stop=True,
                                         skip_group_check=True)
                        rb = p_small.tile([64, TPC], f32, tag="rb", name="rb")
                        nc.vector.tensor_copy(rb[:], rb_ps[0:64, 0:TPC])
                        nc.vector.scalar_tensor_tensor(
                            ot_sb[fi][poff:poff + 64, :],
                            ps_o[hs][0:64, 0:TPC], 1.0 / S_V, rb[:],
                            OP.mult, OP.mult)

                # ======== out-proj + residual
                ow = p_wo.tile([128, 8 * 1024], f8, tag="ow", name="ow")
                nc.sync.dma_start(ow[:], outw.ap()[l])
                for cc in range(2):
                    pss = [pst() for _ in range(2)]
                    for di in range(8):
                        for ti in range(2):
                            nc.tensor.matmul(
                                pss[ti][:], ot_sb[di][:, ti * 128:(ti + 1) * 128],
                                ow[:, di * 1024 + cc * 512:di * 1024 + cc * 512 + 512],
                                start=(di == 0), stop=(di == 7),
                                skip_group_check=True)
                    for ti in range(2):
                        nc.vector.scalar_tensor_tensor(
                            h_sb[ti][:, cc * 512:(cc + 1) * 512],
                            pss[ti][:], inv_s_out,
                            h_sb[ti][:, cc * 512:(cc + 1) * 512],
                            OP.mult, OP.add)

                # ======== LN2 + FFN
                layer_norm_t(y_t)

                if b1_nz:
                    b1_sb = p_small.tile([128, 32], f32, tag="b1sb", name="b1sb")
                    nc.sync.dma_start(b1_sb[:], b1e_p.ap()[l].rearrange("a b -> b a"))
                for ffg in range(8):
                    wt = p_w1.tile([128, 8 * 512], bf16, tag="w1t", name="w1t")
                    nc.sync.dma_start(wt[:], w1p.ap()[l, ffg])
                    pss = [pst() for _ in range(2)]
                    for ci in range(8):
                        for sub in range(4):
                            nc.tensor.matmul(
                                pss[sub // 2][:, (sub % 2) * 256:(sub % 2) * 256 + 256],
                                wt[:, ci * 512 + sub * 128:ci * 512 + sub * 128 + 128],
                                y_t[ci][:],
                                start=(ci == 0 and sub % 2 == 0),
                                stop=(ci == 7 and sub % 2 == 1),
                                skip_group_check=True)
                    if b1_nz:
                        for sub in range(4):
                            ffi = ffg * 4 + sub
                            nc.scalar.activation(
                                h1_t[ffg][:, sub * 256:(sub + 1) * 256],
                                pss[sub // 2][:, (sub % 2) * 256:(sub % 2) * 256 + 256],
                                AF.Gelu, bias=b1_sb[:, ffi:ffi + 1])
                    else:
                        for k2 in range(2):
                            nc.scalar.activation(
                                h1_t[ffg][:, k2 * 512:(k2 + 1) * 512],
                                pss[k2][:, 0:512], AF.Gelu)

                pss2 = [[pst() for _ in range(2)] for _ in range(2)]  # [cc][ti]
                for g in range(8):
                    wt = p_w2.tile([128, 4 * 1024], bf16, tag="w2t", name="w2t")
                    nc.sync.dma_start(wt[:], w2p.ap()[l, g])
                    for f4 in range(4):
                        ffi = g * 4 + f4
                        for cc in range(2):
                            for ti in range(2):
                                nc.tensor.matmul(
                                    pss2[cc][ti][:],
                                    h1_t[ffi // 4][:, (ffi % 4) * 256 + ti * 128:
                                                   (ffi % 4) * 256 + ti * 128 + 128],
                                    wt[:, f4 * 1024 + cc * 512:f4 * 1024 + cc * 512 + 512],
                                    start=(ffi == 0), stop=(ffi == 31),
                                    skip_group_check=True)
                for cc in range(2):
                    if b2_nz:
                        b2_sb = p_small.tile([1, 512], bf16, tag="b2sb", name="b2sb")
                        nc.sync.dma_start(
                            b2_sb[:], b2l_p.ap()[l][:, cc * 512:(cc + 1) * 512])
                        for ti in range(2):
                            nc.tensor.matmul(pss2[cc][ti][:], ones1[:], b2_sb[:],
                                             start=False, stop=True,
                                             skip_group_check=True)
                    for ti in range(2):
                        nc.vector.tensor_tensor(
                            h_sb[ti][:, cc * 512:(cc + 1) * 512],
                            h_sb[ti][:, cc * 512:(cc + 1) * 512], pss2[cc][ti][:],
                            OP.add)

            # ======== head + gate + output
            layer_norm_t(y_t)

            hb1_sb = p_small.tile([128, 4], f32, tag="hb1", name="hb1")
            nc.sync.dma_start(hb1_sb[:], hb1_p.ap().rearrange("a b -> b a"))
            g1_t = [p_small.tile([128, TPC], bf16, tag=f"g1{i}", name=f"g1{i}") for i in range(4)]
            hwt = p_w1.tile([128, 8 * 512], bf16, tag="w1t", name="hw1t")
            nc.sync.dma_start(hwt[:], hw1p.ap()[:, :])
            pss4 = [pst() for _ in range(2)]
            for ci in range(8):
                for sub in range(4):
                    nc.tensor.matmul(
                        pss4[sub // 2][:, (sub % 2) * 256:(sub % 2) * 256 + 256],
                        hwt[:, ci * 512 + sub * 128:ci * 512 + sub * 128 + 128],
                        y_t[ci][:],
                        start=(ci == 0 and sub % 2 == 0),
                        stop=(ci == 7 and sub % 2 == 1),
                        skip_group_check=True)
            for sub in range(4):
                nc.scalar.activation(g1_t[sub][:],
                                     pss4[sub // 2][:, (sub % 2) * 256:(sub % 2) * 256 + 256],
                                     AF.Gelu, bias=hb1_sb[:, sub:sub + 1])

            ps_r = pst()
            for sub in range(4):
                wt = p_small.tile([128, 7], bf16, tag="hw2t", name="hw2t")
                nc.sync.dma_start(wt[:], hw2p.ap()[sub * 128:(sub + 1) * 128, :])
                nc.tensor.matmul(ps_r[0:7, 0:TPC], wt[:], g1_t[sub][:],
                                 start=(sub == 0), stop=(sub == 3),
                                 skip_group_check=True)
            scal_t = p_small.tile([7, TPC], f32, tag="scal", name="scal")
            nc.scalar.activation(scal_t[:], ps_r[0:7, 0:TPC], AF.Sigmoid, bias=hb2_t[:])
            tanh_t = p_small.tile([7, TPC], f32, tag="tanh", name="tanh")
            nc.scalar.activation(tanh_t[:], ps_r[0:7, 0:TPC], AF.Tanh, bias=hb2_t[:])

            for ti in range(2):
                # learned gate: sigmoid(h @ gate_w + gate_b)
                mul_t = p_scr.tile([128, D], f32, tag="scr", name="mul")[:]
                nc.vector.tensor_tensor(mul_t, h_sb[ti][:], gw_b[:], OP.mult)
                lsum = p_stat.tile([128, 1], f32, tag="lsum", name="lsum")
                nc.vector.reduce_sum(lsum[:], mul_t, axis=mybir.AxisListType.X)
                learned = p_stat.tile([128, 1], f32, tag="learned", name="learned")
                nc.scalar.activation(learned[:], lsum[:], AF.Sigmoid,
                                     bias=gb_t[:])
                # scalars natural via PE transpose
                ps_t = pst()
                nc.tensor.transpose(ps_t[:, 0:7],
                                    scal_t[:, ti * 128:(ti + 1) * 128], idf[0:7, 0:7])
                ps_t2 = pst()
                nc.tensor.transpose(ps_t2[:, 0:7],
                                    tanh_t[:, ti * 128:(ti + 1) * 128], idf[0:7, 0:7])
                nc.scalar.copy(out_sb[ti][:, D:D + 7], ps_t[:, 0:7])
                nc.vector.tensor_scalar(out_sb[ti][:, D + 2:D + 3],
                                        ps_t2[:, 2:3], 2.0, None, OP.mult)
                # gate = sigmoid(gc0*learned + gc1*scal0 + gcb)
                gp = p_stat.tile([128, 1], f32, tag="gp", name="gp")
                nc.vector.tensor_scalar(gp[:], learned[:], gc0_c, None, OP.mult)
                gp2 = p_stat.tile([128, 1], f32, tag="gp2", name="gp2")
                nc.vector.tensor_scalar(gp2[:], ps_t[:, 0:1], gc1_c, None,
                                        OP.mult)
                nc.vector.tensor_tensor(gp[:], gp[:], gp2[:], OP.add)
                nc.scalar.activation(out_sb[ti][:, D + 7:D + 8], gp[:], AF.Sigmoid,
                                     bias=gcb_t[:])
                nc.vector.tensor_copy(out_sb[ti][:, 0:D], h_sb[ti][:])
                nc.sync.dma_start(out_p.ap()[ti * 128:(ti + 1) * 128, :],
                                  out_sb[ti][:])
    return nc


def split_drain_waits(nc, mybir, cap=1):
    """Walrus CoreV3 caps sync-wait commands per instruction at one; move
    excess waits onto injected no-ops preceding the instruction."""
    import bass_rust
    for fn in nc.m.functions:
        for bb in fn.blocks:
            changed = False
            new_insts = []
            for inst in bb.instructions:
                si = inst.sync_info
                if (si is not None and si.on_wait and len(si.on_wait) > cap
                        and inst.engine != mybir.EngineType.Unassigned):
                    waits = list(si.on_wait)
                    head, tail = waits[:-cap], waits[-cap:]
                    for i in range(0, len(head), cap):
                        d = mybir.InstNoOp(name=f"{inst.name}_sw{i}", ins=[],
                                           outs=[])
                        d.engine = inst.engine
                        d.sync_info = bass_rust.SyncInfo(
                            on_wait=head[i:i + cap], on_update=[])
                        new_insts.append(d)
                        nc.register_instruction(d, overwrite=True)
                    inst.sync_info = bass_rust.SyncInfo(
                        on_wait=tail, on_update=list(si.on_update or []))
                    changed = True
                new_insts.append(inst)
            if changed:
                bb.instructions[:] = new_insts
    return nc


def _q8(a, target=128.0):
    """Quantize to fp8e4m3 with a power-of-2 scale; returns (q8, inv_scale)."""
    a = np.asarray(a, np.float32)
    am = float(np.abs(a).max())
    s = 2.0 ** np.floor(np.log2(target / am)) if am > 0 else 1.0
    return (a * s).astype(F8), np.float32(1.0 / s)


def _host_prep(inputs, n_layers=L):
    """Fold gains/scale into weights, build per-core shards."""
    f = lambda k: np.asarray(inputs[k], dtype=np.float32)
    x = f('x'); traj = f('trajectory_bias')
    qkv_w = f('qkv_w'); out_w = f('out_w')
    w1 = f('w1'); b1 = f('b1'); w2 = f('w2'); b2 = f('b2')
    ln1_g = f('ln1_g'); ln1_b = f('ln1_b'); ln2_g = f('ln2_g'); ln2_b = f('ln2_b')
    head_ln_g = f('head_ln_g'); head_ln_b = f('head_ln_b')
    head_w1 = f('head_w1'); head_b1 = f('head_b1')
    head_w2 = f('head_w2'); head_b2 = f('head_b2')
    gate_w = f('gate_w'); gate_b = f('gate_b')
    gatec_w = f('gatec_w'); gatec_b = f('gatec_b')

    scale = np.float32(1.0 / np.sqrt(DH))
    colscale = np.concatenate([np.full(D, scale, np.float32),
                               np.ones(2 * D, np.float32)])
    qkv_eff = (ln1_g[:, :, None] * qkv_w) * colscale[None, None, :]
    qkv_bias = np.einsum('lc,lcf->lf', ln1_b, qkv_w * colscale[None, None, :])
    w1_eff = ln2_g[:, :, None] * w1
    b1_eff = b1 + np.einsum('lc,lcf->lf', ln2_b, w1)
    hw1_eff = head_ln_g[:, None] * head_w1
    hb1_eff = head_b1 + head_ln_b @ head_w1

    v_bias = qkv_bias[:, 2 * D:]
    qk_bias = qkv_bias[:, :2 * D]
    qkvb_nz = bool(np.any(qkv_bias != 0))
    v_bias_nz = bool(np.any(v_bias != 0))
    b1_nz = bool(np.any(b1_eff != 0))
    b2_nz = bool(np.any(b2 != 0))

    # fp8 weights
    qkv8, inv_s_qkv = _q8(qkv_eff[:n_layers])
    out8, inv_s_out = _q8(out_w[:n_layers])
    # slab layouts
    #  qkvw [l, fg, p, (ci c)]
    qkv_sl = qkv8.reshape(n_layers, 8, 128, 6, 512).transpose(0, 3, 2, 1, 4)
    qkv_sl = np.ascontiguousarray(qkv_sl.reshape(n_layers, 6, 128, 8 * 512))
    #  outw [l, p, (di cc c)]
    out_sl = out8.reshape(n_layers, 8, 128, 2, 512).transpose(0, 2, 1, 3, 4)
    out_sl = np.ascontiguousarray(out_sl.reshape(n_layers, 128, 8 * 1024))
    #  w1 [l, ffg, p, (ci c)]
    w1_sl = w1_eff[:n_layers].astype(BF16).reshape(n_layers, 8, 128, 8, 512)
    w1_sl = np.ascontiguousarray(w1_sl.transpose(0, 3, 2, 1, 4).reshape(
        n_layers, 8, 128, 8 * 512))
    #  w2 [l, g, p, (f4 c)]
    w2_sl = w2[:n_layers].astype(BF16).reshape(n_layers, 8, 4, 128, 1024)
    w2_sl = np.ascontiguousarray(w2_sl.transpose(0, 1, 3, 2, 4).reshape(
        n_layers, 8, 128, 4 * 1024))
    #  hw1 [p, (ci c)]
    hw1_sl = hw1_eff.astype(BF16).reshape(8, 128, 512).transpose(1, 0, 2)
    hw1_sl = np.ascontiguousarray(hw1_sl.reshape(128, 8 * 512))

    pos = np.arange(S)
    causal = np.where(pos[None, :] <= pos[:, None], 0.0, MASK8).astype(np.float32)
    window = np.where(np.abs(pos[:, None] - pos[None, :]) <= W // 2, 0.0,
                      MASK8).astype(np.float32)

    shared = {
        'qkvw': qkv_sl,
        'outw': out_sl,
        'w1p': w1_sl,
        'w2p': w2_sl,
        'hw1p': hw1_sl,
        'hw2p': head_w2.astype(BF16),
        'gwp': np.ascontiguousarray(
            np.broadcast_to(gate_w.reshape(1, D), (128, D))).astype(np.float32),
        'identf': np.eye(128, dtype=np.float32),
        'identb': np.eye(128, dtype=np.float32).astype(BF16),
        'ident8': np.eye(128, dtype=np.float32).astype(F8),
        'qkvb_p': (qk_bias[:n_layers] / inv_s_qkv).reshape(
            n_layers, 16, 128).astype(np.float32),
        'b1e_p': b1_eff[:n_layers].reshape(n_layers, 32, 128).astype(np.float32),
        'hb1_p': hb1_eff.reshape(4, 128).astype(np.float32),
        'hb2_p': head_b2.reshape(7, 1).astype(np.float32),
        'vbl_p': (v_bias[:n_layers] / inv_s_qkv).reshape(
            n_layers, 1, D).astype(BF16),
        'b2l_p': b2[:n_layers].reshape(n_layers, 1, D).astype(BF16),
    }
    gate_consts = (float(gate_b[0]), float(gatec_w[0, 0]), float(gatec_w[1, 0]),
                   float(gatec_b[0]))

    extra = {'qkvb_nz': qkvb_nz, 'v_bias_nz': v_bias_nz, 'b1_nz': b1_nz,
             'b2_nz': b2_nz, 'gate_consts': gate_consts,
             'inv_s_qkv': float(inv_s_qkv), 'inv_s_out': float(inv_s_out)}

    geoms = {py: _pair_geom(py) for py in (0, 1)}
    in_maps = []
    for c in range(NCORE):
        b, p = c // GROUP, c % GROUP
        gq = LOCAL2GLOBAL[p]
        m = dict(shared)
        m['x_sh'] = np.ascontiguousarray(x[b][gq])
        for py, key in ((0, 'biasp_e'), (1, 'biasp_o')):
            bp = traj[b] + causal + (window if py == 0 else 0.0)  # [H,Sq,Sk]
            sh = bp[:, gq][:, :, KTILDE2GLOBAL]                   # [H,256,1024]
            sh = np.transpose(sh, (2, 0, 1))                      # [k~,H,q~]
            sh = sh.reshape(8, 128, H, TPC)                       # [jt,kp,h,q~]
            c00, c01, c10, c11, W0, Wp = geoms[py]
            eb = np.zeros((8, 128, 4, 2, Wp), np.float32)
            for fi in range(8):
                for p4 in range(4):
                    for hs in range(2):
                        hd = 2 * fi + hs
                        eb[fi, :, p4, hs, 0:W0] = sh[2 * p4, :, hd, c00:c01]
                        eb[fi, :, p4, hs, W0:Wp] = sh[2 * p4 + 1, :, hd, c10:c11]
            m[key] = np.ascontiguousarray(
                np.clip(eb, -240.0, 240.0).reshape(8, 128, 8 * Wp).astype(F8))
        in_maps.append(m)
    return in_maps, extra


def _unshard(results):
    full = np.zeros((B, S, D + 8), np.float32)
    for c in range(NCORE):
        b, p = c // GROUP, c % GROUP
        full[b, LOCAL2GLOBAL[p]] = results[c]['out']
    return full


def kernel(**inputs):
    global LAST_RESULT
    import sys
    for pth in ('/opt/trn_rl_repo', '/opt/pypackages'):
        if pth not in sys.path:
            sys.path.append(pth)
    import concourse.bass as bass
    import concourse.tile as tile
    import concourse.mybir as mybir
    from concourse.bass_utils import run_bass_kernel_spmd

    in_maps, extra = _host_prep(inputs)
    nc = build_nc(bass, tile, mybir, n_layers=L,
                  qkvb_nz=extra['qkvb_nz'], v_bias_nz=extra['v_bias_nz'],
                  b1_nz=extra['b1_nz'], b2_nz=extra['b2_nz'],
                  gate_consts=extra['gate_consts'],
                  inv_s_qkv=extra['inv_s_qkv'], inv_s_out=extra['inv_s_out'])
    split_drain_waits(nc, mybir)
    res = run_bass_kernel_spmd(nc, in_maps, core_ids=list(range(NCORE)))
    LAST_RESULT = res
    return _unshard(res.results)


# revision 22
# speedup vs baseline: 1.1085x; 1.1085x over previous
"""Trainium2 Bass kernel for nn_AnticipatoryTransformer (8 NeuronCores).

v2 strategy (sequence-parallel, self-contained):
  - 2048 tokens (B=2 x S=1024) sharded 8 ways: core c handles batch b=c//4,
    rank p=c%4 of a 4-core group; 32-row striping (256 tokens/core).
  - Per layer: LN1 -> y^T (bf16 matmul transposes) -> K^T first, AllGather-K
    launched early; V next, AllGather-V; Q last (overlaps the collectives).
  - Attention: per head-pair (64-row PE groups run concurrently), per
    k~-tile-pair: scores into a tau-merged [128, Wp] PSUM bank, one merged
    fp8 bias matmul (trajectory+causal+window, host-packed, masked lanes get
    -224 => exp -> 0), one exp per head over the merged tile, AV accumulate
    with an extra ones column yielding softmax denominators.  Normalize via
    reciprocal_approx_fast + gpsimd partition_broadcast + DVE multiply.
  - QKV/out-proj weights + attention bias in fp8e4m3 (dequant scale folded
    into the PSUM->SBUF copies / residual adds), FFN weights bf16.
  - Weight loads as ~0.5-2MB slab DMAs; PSUM managed as one shared 8-bank
    pool; elementwise PSUM drains on DVE; gelu batched [128,512].
"""

import numpy as np
import ml_dtypes

BF16 = ml_dtypes.bfloat16
F8 = ml_dtypes.float8_e4m3
B, S, D, H, DH, L, FF, W = 2, 1024, 1024, 16, 64, 4, 4096, 256
NEG = -1e9
MASK8 = -224.0          # fp8 bias mask; exp(score + MASK8) == 0 in fp32
EPS = 1e-5
GROUP = 4
NCORE = 8
TPC = 256          # tokens per core
VE = 65            # V columns per head incl. ones column
VEXT = H * VE      # 1040

LAST_RESULT = None


def _gtok(rank, t):
    return 32 * (4 * (t // 32) + rank) + t % 32


LOCAL2GLOBAL = {p: np.array([_gtok(p, j) for j in range(TPC)]) for p in range(4)}
KTILDE2GLOBAL = np.array([_gtok(r, t) for r in range(4) for t in range(TPC)])


def _colrange(parity, tau):
    """Active q~ column range for a k~-tile with t-half tau, given layer parity."""
    if parity == 1:  # odd layer: causal only
        return (128 * tau, 256)
    return (max(0, 32 * (4 * tau - 1)), min(256, 32 * (4 * tau + 5)))


def _pair_geom(parity):
    """(c00,c01,c10,c11,W0,Wp) for a tau-merged k~-tile pair."""
    c00, c01 = _colrange(parity, 0)
    c10, c11 = _colrange(parity, 1)
    W0 = c01 - c00
    Wp = W0 + (c11 - c10)
    return c00, c01, c10, c11, W0, Wp


def build_nc(bass, tile, mybir, n_layers=L, qkvb_nz=False, v_bias_nz=False,
             b1_nz=False, b2_nz=False, gate_consts=(0.0, 1.0, 1.0, 0.0),
             inv_s_qkv=1.0, inv_s_out=1.0):
    """Build the SPMD Bass graph (identical on all 8 cores)."""
    gate_b_c, gc0_c, gc1_c, gcb_c = (float(v) for v in gate_consts)
    from contextlib import ExitStack

    dt = mybir.dt
    AF = mybir.ActivationFunctionType
    OP = mybir.AluOpType

    nc = bass.Bass("TRN2", target_bir_lowering=False, debug=False,
                   num_devices=NCORE)

    f32, bf16, f8 = dt.float32, dt.bfloat16, dt.float8e4
    din = lambda name, shape, d: nc.dram_tensor(name, shape, d, kind="ExternalInput")

    WP = [_pair_geom(0)[5], _pair_geom(1)[5]]      # 320, 384
    S_K = 8.0   # K stored as fp8 * S_K; descaled via the Q copy
    S_V = 8.0   # V stored as fp8 * S_V; descaled in the epilogue multiply

    x_in = din("x_sh", [TPC, D], f32)
    biasp_e = din("biasp_e", [8, 128, 8 * WP[0]], f8)
    biasp_o = din("biasp_o", [8, 128, 8 * WP[1]], f8)
    qkvw = din("qkvw", [n_layers, 6, 128, 8 * 512], f8)     # [l,fg,p,(ci c)]
    outw = din("outw", [n_layers, 128, 8 * 1024], f8)       # [l,p,(di cc c)]
    w1p = din("w1p", [n_layers, 8, 128, 8 * 512], bf16)     # [l,ffg,p,(ci c)]
    w2p = din("w2p", [n_layers, 8, 128, 4 * 1024], bf16)    # [l,g,p,(f4 c)]
    hw1p = din("hw1p", [128, 8 * 512], bf16)                # [p,(ci c)]
    hw2p = din("hw2p", [D // 2, 7], bf16)
    gwp = din("gwp", [128, D], f32)
    identf = din("identf", [128, 128], f32)
    identb = din("identb", [128, 128], bf16)
    ident8 = din("ident8", [128, 128], f8)
    qkvb_p = din("qkvb_p", [n_layers, 16, 128], f32)     # per-partition bias (xS)
    b1e_p = din("b1e_p", [n_layers, 32, 128], f32)
    hb1_p = din("hb1_p", [4, 128], f32)
    hb2_p = din("hb2_p", [7, 1], f32)
    vbl_p = din("vbl_p", [n_layers, 1, D], bf16)
    b2l_p = din("b2l_p", [n_layers, 1, D], bf16)

    out_p = nc.dram_tensor("out", [TPC, D + 8], f32, kind="ExternalOutput")

    KSZ = 8 * 128 * TPC
    VSZ = 2 * 128 * VEXT
    cc_kin = nc.dram_tensor("cc_kin", [KSZ], f8)
    cc_kout = nc.dram_tensor("cc_kout", [GROUP * KSZ], f8)
    cc_vin = nc.dram_tensor("cc_vin", [VSZ], f8)
    cc_vout = nc.dram_tensor("cc_vout", [GROUP * VSZ], f8)
    rgroups = [[0, 1, 2, 3], [4, 5, 6, 7]]

    with tile.TileContext(nc) as tc:
        with ExitStack() as ctx:
            pool = lambda name, bufs: ctx.enter_context(tc.tile_pool(name=name, bufs=bufs))
            p_const = pool("const", 1)
            p_h = pool("h", 1)
            p_ybf = pool("ybf", 2)
            p_yt = pool("yt", 1)
            p_qt = pool("qt", 1)
            p_ktl = pool("ktl", 1)
            p_vx = pool("vx", 1)
            p_ktf = pool("ktf", 1)
            p_vf = pool("vf", 1)
            p_h1 = pool("h1", 1)
            p_ot = pool("ot", 1)
            p_wq = pool("wq", 3)
            p_wo = pool("wo", 2)
            p_w1 = pool("w1", 4)
            p_w2 = pool("w2", 3)
            p_eb = pool("eb", 2)
            p_at = pool("at", 6)
            p_small = pool("small", 2)
            p_scr = pool("scr", 2)
            p_stat = pool("stat", 8)
            p_outsb = pool("outsb", 1)
            p_ps = ctx.enter_context(tc.tile_pool(name="ps", bufs=8, space="PSUM"))

            def pst():
                return p_ps.tile([128, 512], f32, tag="ps", name="ps")

            # ---- persistent tiles
            h_sb = [p_h.tile([128, D], f32, tag=f"h{i}", name=f"h{i}") for i in range(2)]
            y_t = [p_yt.tile([128, TPC], bf16, tag=f"yt{i}", name=f"yt{i}") for i in range(8)]
            qt_sb = p_qt.tile([128, 8 * TPC], bf16, tag="qt", name="qt")
            ktl_sb = p_ktl.tile([128, 8 * TPC], f8, tag="ktl", name="ktl")
            vx_l = [p_vx.tile([128, VEXT], f8, tag=f"vx{i}", name=f"vx{i}") for i in range(2)]
            kt_f = [p_ktf.tile([128, 4 * TPC], f8, tag=f"ktf{i}", name=f"ktf{i}") for i in range(8)]
            v_f = [p_vf.tile([128, VEXT], f8, tag=f"vf{i}", name=f"vf{i}") for i in range(8)]
            h1_t = [p_h1.tile([128, 4 * TPC], bf16, tag=f"h1{i}", name=f"h1{i}") for i in range(8)]
            ot_sb = [p_ot.tile([128, TPC], bf16, tag=f"ot{i}", name=f"ot{i}") for i in range(8)]
            idf = p_const.tile([128, 128], f32, tag="idf", name="idf")
            idb = p_const.tile([128, 128], bf16, tag="idb", name="idb")
            id8 = p_const.tile([128, 128], f8, tag="id8", name="id8")
            ones1 = p_const.tile([1, 128], bf16, tag="ones1", name="ones1")
            ones1f = p_const.tile([1, 128], f32, tag="ones1f", name="ones1f")
            gw_b = p_const.tile([128, D], f32, tag="gwb", name="gwb")
            hb2_t = p_const.tile([7, 1], f32, tag="hb2", name="hb2")
            eps_t = p_const.tile([128, 1], f32, tag="epst", name="epst")
            gb_t = p_const.tile([128, 1], f32, tag="gbt", name="gbt")
            gcb_t = p_const.tile([128, 1], f32, tag="gcbt", name="gcbt")
            out_sb = [p_outsb.tile([128, D + 8], f32, tag=f"osb{i}", name=f"osb{i}") for i in range(2)]

            # ---- init
            nc.sync.dma_start(idf[:], identf.ap()[:, :])
            nc.sync.dma_start(idb[:], identb.ap()[:, :])
            nc.sync.dma_start(id8[:], ident8.ap()[:, :])
            nc.sync.dma_start(gw_b[:], gwp.ap()[:, :])
            nc.sync.dma_start(hb2_t[:], hb2_p.ap()[:, :])
            nc.vector.memset(ones1[:], 1.0)
            nc.vector.memset(ones1f[:], 1.0)
            nc.vector.memset(eps_t[:], EPS)
            nc.vector.memset(gb_t[:], gate_b_c)
            nc.vector.memset(gcb_t[:], gcb_c)
            for ti in range(2):
                nc.sync.dma_start(h_sb[ti][:], x_in.ap()[ti * 128:(ti + 1) * 128, :])
                ones_ap = vx_l[ti].rearrange("p (h e) -> p h e", e=VE)[:, :, 64:65]
                nc.gpsimd.memset(ones_ap, 1.0)

            def layer_norm_t(dst8):
                """LN of h_sb -> bf16 y (2x[128,D]) -> y^T into dst8 (8x[128,256])."""
                y_bf = [p_ybf.tile([128, D], bf16, tag=f"ybf{i}", name=f"ybf{i}")
                        for i in range(2)]
                for ti in range(2):
                    sq = p_scr.tile([128, D], f32, tag="scr", name="sq")[:]
                    ssum = p_stat.tile([128, 1], f32, tag="ssum", name="ssum")
                    sumsq = p_stat.tile([128, 1], f32, tag="sumsq", name="sumsq")
                    mean = p_stat.tile([128, 1], f32, tag="mean", name="mean")
                    ex2 = p_stat.tile([128, 1], f32, tag="ex2", name="ex2")
                    msq = p_stat.tile([128, 1], f32, tag="msq", name="msq")
                    var = p_stat.tile([128, 1], f32, tag="var", name="var")
                    std = p_stat.tile([128, 1], f32, tag="std", name="std")
                    istd = p_stat.tile([128, 1], f32, tag="istd", name="istd")
                    nc.scalar.activation(sq, h_sb[ti][:], AF.Square,
                                         accum_out=sumsq[:])
                    nc.vector.reduce_sum(ssum[:], h_sb[ti][:], axis=mybir.AxisListType.X)
                    nc.vector.tensor_scalar(mean[:], ssum[:], 1.0 / D, None, OP.mult)
                    nc.vector.tensor_scalar(ex2[:], sumsq[:], 1.0 / D, None, OP.mult)
                    nc.vector.tensor_tensor(msq[:], mean[:], mean[:], OP.mult)
                    nc.vector.tensor_tensor(var[:], ex2[:], msq[:], OP.subtract)
                    nc.scalar.activation(std[:], var[:], AF.Sqrt, bias=eps_t[:])
                    nc.vector.reciprocal(istd[:], std[:])
                    nc.vector.tensor_scalar(y_bf[ti][:], h_sb[ti][:],
                                            mean[:], istd[:], OP.subtract, OP.mult)
                for ci in range(8):
                    ps = pst()
                    for ti in range(2):
                        nc.tensor.matmul(ps[:, ti * 128:(ti + 1) * 128],
                                         y_bf[ti][:, ci * 128:(ci + 1) * 128],
                                         idb[:], start=(ti == 0), stop=(ti == 1),
                                         skip_group_check=True)
                    nc.vector.tensor_copy(dst8[ci][:], ps[:, 0:TPC])

            for l in range(n_layers):
                parity = l % 2
                bias_dram = biasp_o if parity else biasp_e
                c00, c01, c10, c11, W0, Wp = _pair_geom(parity)

                # ======== LN1 + y1^T
                layer_norm_t(y_t)

                if qkvb_nz:
                    qkvb_sb = p_small.tile([128, 16], f32, tag="qkvb", name="qkvb")
                    nc.sync.dma_start(
                        qkvb_sb[:], qkvb_p.ap()[l].rearrange("a b -> b a"))

                # ======== QKV in order K, V, Q with early collectives
                def qk_block(fg, dst, dst_off, scl):
                    wt = p_wq.tile([128, 8 * 512], f8, tag="wq", name="wq")
                    nc.sync.dma_start(wt[:], qkvw.ap()[l, fg])
                    pss = [pst() for _ in range(2)]
                    for ci in range(8):
                        for sub in range(4):
                            nc.tensor.matmul(
                                pss[sub // 2][:, (sub % 2) * 256:(sub % 2) * 256 + 256],
                                wt[:, ci * 512 + sub * 128: ci * 512 + sub * 128 + 128],
                                y_t[ci][:],
                                start=(ci == 0 and sub % 2 == 0),
                                stop=(ci == 7 and sub % 2 == 1),
                                skip_group_check=True)
                    for sub in range(4):
                        fi = fg * 4 + sub - dst_off
                        if qkvb_nz:
                            dsl = dst[:, fi * 256:(fi + 1) * 256]
                            srcp = pss[sub // 2][:, (sub % 2) * 256:(sub % 2) * 256 + 256]
                            nc.vector.tensor_scalar(
                                dsl, srcp, qkvb_sb[:, (fg * 4 + sub):(fg * 4 + sub) + 1],
                                scl, OP.add, OP.mult)
                        elif sub % 2 == 0:
                            fi0 = fg * 4 + sub - dst_off
                            dsl = dst[:, fi0 * 256:(fi0 + 2) * 256]
                            nc.vector.tensor_scalar(dsl, pss[sub // 2][:, 0:512],
                                                    scl, None, OP.mult)

                # K (features 1024:2048 -> fg 2,3)
                for fg in (2, 3):
                    qk_block(fg, ktl_sb, 8, inv_s_qkv * S_K)
                nc.sync.dma_start(
                    cc_kin.ap().rearrange("(f p t) -> p f t", p=128, t=TPC),
                    ktl_sb.rearrange("p (f t) -> p f t", t=TPC))
                nc.gpsimd.collective_compute(
                    "AllGather", mybir.AluOpType.bypass, replica_groups=rgroups,
                    ins=[cc_kin.ap().opt()], outs=[cc_kout.ap().opt()])
                cko = cc_kout.ap().rearrange("(r f p t) -> f p r t", r=GROUP, f=8,
                                             p=128)
                for fi in range(8):
                    nc.sync.dma_start(kt_f[fi][:], cko[fi])

                # Q (fg 0,1)
                for fg in (0, 1):
                    qk_block(fg, qt_sb, 0, inv_s_qkv / S_K)

                # V (fg 4,5)
                for fg in (4, 5):
                    pss = [pst() for _ in range(2)]
                    wt = p_wq.tile([128, 8 * 512], f8, tag="wq", name="wq")
                    nc.sync.dma_start(wt[:], qkvw.ap()[l, fg])
                    for ci in range(8):
                        for ti in range(2):
                            nc.tensor.matmul(
                                pss[ti][:], y_t[ci][:, ti * 128:(ti + 1) * 128],
                                wt[:, ci * 512:(ci + 1) * 512],
                                start=(ci == 0), stop=(ci == 7),
                                skip_group_check=True)
                    if v_bias_nz:
                        vb_sb = p_small.tile([1, 512], bf16, tag="vb", name="vb")
                        nc.sync.dma_start(
                            vb_sb[:], vbl_p.ap()[l][:, (fg - 4) * 512:(fg - 3) * 512])
                        for ti in range(2):
                            nc.tensor.matmul(pss[ti][:], ones1[:], vb_sb[:],
                                             start=False, stop=True,
                                             skip_group_check=True)
                    h0 = (fg - 4) * 8
                    for ti in range(2):
                        dstv = vx_l[ti].rearrange("p (h e) -> p h e", e=VE)[
                            :, h0:h0 + 8, 0:64]
                        nc.scalar.activation(
                            dstv, pss[ti].rearrange("p (h e) -> p h e", e=64),
                            AF.Copy, scale=inv_s_qkv * S_V)
                for ti in range(2):
                    nc.sync.dma_start(
                        cc_vin.ap().rearrange("(a p f) -> a p f", a=2, p=128)[ti],
                        vx_l[ti][:])
                nc.gpsimd.collective_compute(
                    "AllGather", mybir.AluOpType.bypass, replica_groups=rgroups,
                    ins=[cc_vin.ap().opt()], outs=[cc_vout.ap().opt()])
                cvo = cc_vout.ap().rearrange("(r a p f) -> r a p f", r=GROUP, a=2,
                                             p=128)
                for jt in range(8):
                    nc.sync.dma_start(v_f[jt][:], cvo[jt // 2, jt % 2])

                # ======== attention
                for fi in range(8):
                    eb = p_eb.tile([128, 8 * Wp], f8, tag="eb", name="eb")
                    nc.sync.dma_start(eb[:], bias_dram.ap()[fi])
                    ps_o = [pst(), pst()]
                    for p4 in range(4):
                        jt0, jt1 = 2 * p4, 2 * p4 + 1
                        ps_s = [pst(), pst()]
                        for hs in range(2):
                            poff = hs * 64
                            nc.tensor.matmul(
                                ps_s[hs][:, 0:W0],
                                kt_f[fi][poff:poff + 64, jt0 * 128:jt0 * 128 + 128],
                                qt_sb[poff:poff + 64, fi * 256 + c00:fi * 256 + c01],
                                start=True, stop=False, skip_group_check=True)
                        for hs in range(2):
                            poff = hs * 64
                            nc.tensor.matmul(
                                ps_s[hs][:, W0:Wp],
                                kt_f[fi][poff:poff + 64, jt1 * 128:jt1 * 128 + 128],
                                qt_sb[poff:poff + 64, fi * 256 + c10:fi * 256 + c11],
                                start=False, stop=False, skip_group_check=True)
                        for hs in range(2):
                            nc.tensor.matmul(
                                ps_s[hs][:, 0:Wp], id8[:],
                                eb[:, (p4 * 2 + hs) * Wp:(p4 * 2 + hs + 1) * Wp],
                                start=False, stop=True, skip_group_check=True)
                        for hs in range(2):
                            at = p_at.tile([128, 384], bf16, tag="at",
                                           name="at")
                            nc.scalar.activation(at[:, 0:Wp], ps_s[hs][:, 0:Wp],
                                                 AF.Exp)
                            hd = 2 * fi + hs
                            nc.tensor.matmul(
                                ps_o[hs][0:VE, c00:c01],
                                v_f[jt0][:, hd * VE:(hd + 1) * VE],
                                at[:, 0:W0],
                                start=(p4 == 0), stop=False, skip_group_check=True)
                            nc.tensor.matmul(
                                ps_o[hs][0:VE, c10:c11],
                                v_f[jt1][:, hd * VE:(hd + 1) * VE],
                                at[:, W0:Wp],
                                start=False, stop=(p4 == 3), skip_group_check=True)
                    for hs in range(2):
                        poff = hs * 64
                        dn = p_small.tile([1, TPC], f32, tag="dn", name="dn")
                        nc.vector.reciprocal(dn[:], ps_o[hs][64:65, 0:TPC])
                        rb_ps = pst()
                        nc.tensor.matmul(rb_ps[0:64, 0:TPC], ones1f[0:1, 0:64],
                                         dn[:], start=True, stop=True,
                                         skip_group_check=True)
                        rb = p_small.tile([64, TPC], f32, tag="rb", name="rb")
                        nc.vector.tensor_copy(rb[:], rb_ps[0:64, 0:TPC])
                        nc.vector.scalar_tensor_tensor(
                            ot_sb[fi][poff:poff + 64, :],
                            ps_o[hs][0:64, 0:TPC], 1.0 / S_V, rb[:],
                            OP.mult, OP.mult)

                # ======== out-proj + residual
                ow = p_wo.tile([128, 8 * 1024], f8, tag="ow", name="ow")
                nc.sync.dma_start(ow[:], outw.ap()[l])
                for cc in range(2):
                    pss = [pst() for _ in range(2)]
                    for di in range(8):
                        for ti in range(2):
                            nc.tensor.matmul(
                                pss[ti][:], ot_sb[di][:, ti * 128:(ti + 1) * 128],
                                ow[:, di * 1024 + cc * 512:di * 1024 + cc * 512 + 512],
                                start=(di == 0), stop=(di == 7),
                                skip_group_check=True)
                    for ti in range(2):
                        nc.vector.scalar_tensor_tensor(
                            h_sb[ti][:, cc * 512:(cc + 1) * 512],
                            pss[ti][:], inv_s_out,
                            h_sb[ti][:, cc * 512:(cc + 1) * 512],
                            OP.mult, OP.add)

                # ======== LN2 + FFN
                layer_norm_t(y_t)

                if b1_nz:
                    b1_sb = p_small.tile([128, 32], f32, tag="b1sb", name="b1sb")
                    nc.sync.dma_start(b1_sb[:], b1e_p.ap()[l].rearrange("a b -> b a"))
                for ffg in range(8):
                    wt = p_w1.tile([128, 8 * 512], bf16, tag="w1t", name="w1t")
                    nc.sync.dma_start(wt[:], w1p.ap()[l, ffg])
                    pss = [pst() for _ in range(2)]
                    for ci in range(8):
                        for sub in range(4):
                            nc.tensor.matmul(
                                pss[sub // 2][:, (sub % 2) * 256:(sub % 2) * 256 + 256],
                                wt[:, ci * 512 + sub * 128:ci * 512 + sub * 128 + 128],
                                y_t[ci][:],
                                start=(ci == 0 and sub % 2 == 0),
                                stop=(ci == 7 and sub % 2 == 1),
                                skip_group_check=True)
                    if b1_nz:
                        for sub in range(4):
                            ffi = ffg * 4 + sub
                            nc.scalar.activation(
                                h1_t[ffg][:, sub * 256:(sub + 1) * 256],
                                pss[sub // 2][:, (sub % 2) * 256:(sub % 2) * 256 + 256],
                                AF.Gelu, bias=b1_sb[:, ffi:ffi + 1])
                    else:
                        for k2 in range(2):
                            nc.scalar.activation(
                                h1_t[ffg][:, k2 * 512:(k2 + 1) * 512],
                                pss[k2][:, 0:512], AF.Gelu)

                pss2 = [[pst() for _ in range(2)] for _ in range(2)]  # [cc][ti]
                for g in range(8):
                    wt = p_w2.tile([128, 4 * 1024], bf16, tag="w2t", name="w2t")
                    nc.sync.dma_start(wt[:], w2p.ap()[l, g])
                    for f4 in range(4):
                        ffi = g * 4 + f4
                        for cc in range(2):
                            for ti in range(2):
                                nc.tensor.matmul(
                                    pss2[cc][ti][:],
                                    h1_t[ffi // 4][:, (ffi % 4) * 256 + ti * 128:
                                                   (ffi % 4) * 256 + ti * 128 + 128],
                                    wt[:, f4 * 1024 + cc * 512:f4 * 1024 + cc * 512 + 512],
                                    start=(ffi == 0), stop=(ffi == 31),
                                    skip_group_check=True)
                for cc in range(2):
                    if b2_nz:
                        b2_sb = p_small.tile([1, 512], bf16, tag="b2sb", name="b2sb")
                        nc.sync.dma_start(
                            b2_sb[:], b2l_p.ap()[l][:, cc * 512:(cc + 1) * 512])
                        for ti in range(2):
                            nc.tensor.matmul(pss2[cc][ti][:], ones1[:], b2_sb[:],
                                             start=False, stop=True,
                                             skip_group_check=True)
                    for ti in range(2):
                        nc.vector.tensor_tensor(
                            h_sb[ti][:, cc * 512:(cc + 1) * 512],
                            h_sb[ti][:, cc * 512:(cc + 1) * 512], pss2[cc][ti][:],
                            OP.add)

            # ======== head + gate + output
            layer_norm_t(y_t)

            hb1_sb = p_small.tile([128, 4], f32, tag="hb1", name="hb1")
            nc.sync.dma_start(hb1_sb[:], hb1_p.ap().rearrange("a b -> b a"))
            g1_t = [p_small.tile([128, TPC], bf16, tag=f"g1{i}", name=f"g1{i}") for i in range(4)]
            hwt = p_w1.tile([128, 8 * 512], bf16, tag="w1t", name="hw1t")
            nc.sync.dma_start(hwt[:], hw1p.ap()[:, :])
            pss4 = [pst() for _ in range(2)]
            for ci in range(8):
                for sub in range(4):
                    nc.tensor.matmul(
                        pss4[sub // 2][:, (sub % 2) * 256:(sub % 2) * 256 + 256],
                        hwt[:, ci * 512 + sub * 128:ci * 512 + sub * 128 + 128],
                        y_t[ci][:],
                        start=(ci == 0 and sub % 2 == 0),
                        stop=(ci == 7 and sub % 2 == 1),
                        skip_group_check=True)
            for sub in range(4):
                nc.scalar.activation(g1_t[sub][:],
                                     pss4[sub // 2][:, (sub % 2) * 256:(sub % 2) * 256 + 256],
                                     AF.Gelu, bias=hb1_sb[:, sub:sub + 1])

            ps_r = pst()
            for sub in range(4):
                wt = p_small.tile([128, 7], bf16, tag="hw2t", name="hw2t")
                nc.sync.dma_start(wt[:], hw2p.ap()[sub * 128:(sub + 1) * 128, :])
                nc.tensor.matmul(ps_r[0:7, 0:TPC], wt[:], g1_t[sub][:],
                                 start=(sub == 0), stop=(sub == 3),
                                 skip_group_check=True)
            scal_t = p_small.tile([7, TPC], f32, tag="scal", name="scal")
            nc.scalar.activation(scal_t[:], ps_r[0:7, 0:TPC], AF.Sigmoid, bias=hb2_t[:])
            tanh_t = p_small.tile([7, TPC], f32, tag="tanh", name="tanh")
            nc.scalar.activation(tanh_t[:], ps_r[0:7, 0:TPC], AF.Tanh, bias=hb2_t[:])

            for ti in range(2):
                # learned gate: sigmoid(h @ gate_w + gate_b)
                mul_t = p_scr.tile([128, D], f32, tag="scr", name="mul")[:]
                nc.vector.tensor_tensor(mul_t, h_sb[ti][:], gw_b[:], OP.mult)
                lsum = p_stat.tile([128, 1], f32, tag="lsum", name="lsum")
                nc.vector.reduce_sum(lsum[:], mul_t, axis=mybir.AxisListType.X)
                learned = p_stat.tile([128, 1], f32, tag="learned", name="learned")
                nc.scalar.activation(learned[:], lsum[:], AF.Sigmoid,
                                     bias=gb_t[:])
                # scalars natural via PE transpose
                ps_t = pst()
                nc.tensor.transpose(ps_t[:, 0:7],
                                    scal_t[:, ti * 128:(ti + 1) * 128], idf[0:7, 0:7])
                ps_t2 = pst()
                nc.tensor.transpose(ps_t2[:, 0:7],
                                    tanh_t[:, ti * 128:(ti + 1) * 128], idf[0:7, 0:7])
                nc.scalar.copy(out_sb[ti][:, D:D + 7], ps_t[:, 0:7])
                nc.vector.tensor_scalar(out_sb[ti][:, D + 2:D + 3],
                                        ps_t2[:, 2:3], 2.0, None, OP.mult)
                # gate = sigmoid(gc0*learned + gc1*scal0 + gcb)
                gp = p_stat.tile([128, 1], f32, tag="gp", name="gp")
                nc.vector.tensor_scalar(gp[:], learned[:], gc0_c, None, OP.mult)
                gp2 = p_stat.tile([128, 1], f32, tag="gp2", name="gp2")
                nc.vector.tensor_scalar(gp2[:], ps_t[:, 0:1], gc1_c, None,
                                        OP.mult)
                nc.vector.tensor_tensor(gp[:], gp[:], gp2[:], OP.add)
                nc.scalar.activation(out_sb[ti][:, D + 7:D + 8], gp[:], AF.Sigmoid,
                                     bias=gcb_t[:])
                nc.vector.tensor_copy(out_sb[ti][:, 0:D], h_sb[ti][:])
                nc.sync.dma_start(out_p.ap()[ti * 128:(ti + 1) * 128, :],
                                  out_sb[ti][:])
    return nc


def split_drain_waits(nc, mybir, cap=1):
    """Walrus CoreV3 caps sync-wait commands per instruction at one; move
    excess waits onto injected no-ops preceding the instruction."""
    import bass_rust
    for fn in nc.m.functions:
        for bb in fn.blocks:
            changed = False
            new_insts = []
            for inst in bb.instructions:
                si = inst.sync_info
                if (si is not None and si.on_wait and len(si.on_wait) > cap
                        and inst.engine != mybir.EngineType.Unassigned):
                    waits = list(si.on_wait)
                    head, tail = waits[:-cap], waits[-cap:]
                    for i in range(0, len(head), cap):
                        d = mybir.InstNoOp(name=f"{inst.name}_sw{i}", ins=[],
                                           outs=[])
                        d.engine = inst.engine
                        d.sync_info = bass_rust.SyncInfo(
                            on_wait=head[i:i + cap], on_update=[])
                        new_insts.append(d)
                        nc.register_instruction(d, overwrite=True)
                    inst.sync_info = bass_rust.SyncInfo(
                        on_wait=tail, on_update=list(si.on_update or []))
                    changed = True
                new_insts.append(inst)
            if changed:
                bb.instructions[:] = new_insts
    return nc


def _q8(a, target=128.0):
    """Quantize to fp8e4m3 with a power-of-2 scale; returns (q8, inv_scale)."""
    a = np.asarray(a, np.float32)
    am = float(np.abs(a).max())
    s = 2.0 ** np.floor(np.log2(target / am)) if am > 0 else 1.0
    return (a * s).astype(F8), np.float32(1.0 / s)


def _host_prep(inputs, n_layers=L):
    """Fold gains/scale into weights, build per-core shards."""
    f = lambda k: np.asarray(inputs[k], dtype=np.float32)
    x = f('x'); traj = f('trajectory_bias')
    qkv_w = f('qkv_w'); out_w = f('out_w')
    w1 = f('w1'); b1 = f('b1'); w2 = f('w2'); b2 = f('b2')
    ln1_g = f('ln1_g'); ln1_b = f('ln1_b'); ln2_g = f('ln2_g'); ln2_b = f('ln2_b')
    head_ln_g = f('head_ln_g'); head_ln_b = f('head_ln_b')
    head_w1 = f('head_w1'); head_b1 = f('head_b1')
    head_w2 = f('head_w2'); head_b2 = f('head_b2')
    gate_w = f('gate_w'); gate_b = f('gate_b')
    gatec_w = f('gatec_w'); gatec_b = f('gatec_b')

    scale = np.float32(1.0 / np.sqrt(DH))
    colscale = np.concatenate([np.full(D, scale, np.float32),
                               np.ones(2 * D, np.float32)])
    qkv_eff = (ln1_g[:, :, None] * qkv_w) * colscale[None, None, :]
    qkv_bias = np.einsum('lc,lcf->lf', ln1_b, qkv_w * colscale[None, None, :])
    w1_eff = ln2_g[:, :, None] * w1
    b1_eff = b1 + np.einsum('lc,lcf->lf', ln2_b, w1)
    hw1_eff = head_ln_g[:, None] * head_w1
    hb1_eff = head_b1 + head_ln_b @ head_w1

    v_bias = qkv_bias[:, 2 * D:]
    qk_bias = qkv_bias[:, :2 * D]
    qkvb_nz = bool(np.any(qkv_bias != 0))
    v_bias_nz = bool(np.any(v_bias != 0))
    b1_nz = bool(np.any(b1_eff != 0))
    b2_nz = bool(np.any(b2 != 0))

    # fp8 weights
    qkv8, inv_s_qkv = _q8(qkv_eff[:n_layers])
    out8, inv_s_out = _q8(out_w[:n_layers])
    # slab layouts
    #  qkvw [l, fg, p, (ci c)]
    qkv_sl = qkv8.reshape(n_layers, 8, 128, 6, 512).transpose(0, 3, 2, 1, 4)
    qkv_sl = np.ascontiguousarray(qkv_sl.reshape(n_layers, 6, 128, 8 * 512))
    #  outw [l, p, (di cc c)]
    out_sl = out8.reshape(n_layers, 8, 128, 2, 512).transpose(0, 2, 1, 3, 4)
    out_sl = np.ascontiguousarray(out_sl.reshape(n_layers, 128, 8 * 1024))
    #  w1 [l, ffg, p, (ci c)]
    w1_sl = w1_eff[:n_layers].astype(BF16).reshape(n_layers, 8, 128, 8, 512)
    w1_sl = np.ascontiguousarray(w1_sl.transpose(0, 3, 2, 1, 4).reshape(
        n_layers, 8, 128, 8 * 512))
    #  w2 [l, g, p, (f4 c)]
    w2_sl = w2[:n_layers].astype(BF16).reshape(n_layers, 8, 4, 128, 1024)
    w2_sl = np.ascontiguousarray(w2_sl.transpose(0, 1, 3, 2, 4).reshape(
        n_layers, 8, 128, 4 * 1024))
    #  hw1 [p, (ci c)]
    hw1_sl = hw1_eff.astype(BF16).reshape(8, 128, 512).transpose(1, 0, 2)
    hw1_sl = np.ascontiguousarray(hw1_sl.reshape(128, 8 * 512))

    pos = np.arange(S)
    causal = np.where(pos[None, :] <= pos[:, None], 0.0, MASK8).astype(np.float32)
    window = np.where(np.abs(pos[:, None] - pos[None, :]) <= W // 2, 0.0,
                      MASK8).astype(np.float32)

    shared = {
        'qkvw': qkv_sl,
        'outw': out_sl,
        'w1p': w1_sl,
        'w2p': w2_sl,
        'hw1p': hw1_sl,
        'hw2p': head_w2.astype(BF16),
        'gwp': np.ascontiguousarray(
            np.broadcast_to(gate_w.reshape(1, D), (128, D))).astype(np.float32),
        'identf': np.eye(128, dtype=np.float32),
        'identb': np.eye(128, dtype=np.float32).astype(BF16),
        'ident8': np.eye(128, dtype=np.float32).astype(F8),
        'qkvb_p': (qk_bias[:n_layers] / inv_s_qkv).reshape(
            n_layers, 16, 128).astype(np.float32),
        'b1e_p': b1_eff[:n_layers].reshape(n_layers, 32, 128).astype(np.float32),
        'hb1_p': hb1_eff.reshape(4, 128).astype(np.float32),
        'hb2_p': head_b2.reshape(7, 1).astype(np.float32),
        'vbl_p': (v_bias[:n_layers] / inv_s_qkv).reshape(
            n_layers, 1, D).astype(BF16),
        'b2l_p': b2[:n_layers].reshape(n_layers, 1, D).astype(BF16),
    }
    gate_consts = (float(gate_b[0]), float(gatec_w[0, 0]), float(gatec_w[1, 0]),
                   float(gatec_b[0]))

    extra = {'qkvb_nz': qkvb_nz, 'v_bias_nz': v_bias_nz, 'b1_nz': b1_nz,
             'b2_nz': b2_nz, 'gate_consts': gate_consts,
             'inv_s_qkv': float(inv_s_qkv), 'inv_s_out': float(inv_s_out)}

    geoms = {py: _pair_geom(py) for py in (0, 1)}
    in_maps = []
    for c in range(NCORE):
        b, p = c // GROUP, c % GROUP
        gq = LOCAL2GLOBAL[p]
        m = dict(shared)
        m['x_sh'] = np.ascontiguousarray(x[b][gq])
        for py, key in ((0, 'biasp_e'), (1, 'biasp_o')):
            bp = traj[b] + causal + (window if py == 0 else 0.0)  # [H,Sq,Sk]
            sh = bp[:, gq][:, :, KTILDE2GLOBAL]                   # [H,256,1024]
            sh = np.transpose(sh, (2, 0, 1))                      # [k~,H,q~]
            sh = sh.reshape(8, 128, H, TPC)                       # [jt,kp,h,q~]
            c00, c01, c10, c11, W0, Wp = geoms[py]
            eb = np.zeros((8, 128, 4, 2, Wp), np.float32)
            for fi in range(8):
                for p4 in range(4):
                    for hs in range(2):
                        hd = 2 * fi + hs
                        eb[fi, :, p4, hs, 0:W0] = sh[2 * p4, :, hd, c00:c01]
                        eb[fi, :, p4, hs, W0:Wp] = sh[2 * p4 + 1, :, hd, c10:c11]
            m[key] = np.ascontiguousarray(
                np.clip(eb, -240.0, 240.0).reshape(8, 128, 8 * Wp).astype(F8))
        in_maps.append(m)
    return in_maps, extra


def _unshard(results):
    full = np.zeros((B, S, D + 8), np.float32)
    for c in range(NCORE):
        b, p = c // GROUP, c % GROUP
        full[b, LOCAL2GLOBAL[p]] = results[c]['out']
    return full


def kernel(**inputs):
    global LAST_RESULT
    import sys
    for pth in ('/opt/trn_rl_repo', '/opt/pypackages'):
        if pth not in sys.path:
            sys.path.append(pth)
    import concourse.bass as bass
    import concourse.tile as tile
    import concourse.mybir as mybir
    from concourse.bass_utils import run_bass_kernel_spmd

    in_maps, extra = _host_prep(inputs)
    nc = build_nc(bass, tile, mybir, n_layers=L,
                  qkvb_nz=extra['qkvb_nz'], v_bias_nz=extra['v_bias_nz'],
                  b1_nz=extra['b1_nz'], b2_nz=extra['b2_nz'],
                  gate_consts=extra['gate_consts'],
                  inv_s_qkv=extra['inv_s_qkv'], inv_s_out=extra['inv_s_out'])
    split_drain_waits(nc, mybir)
    res = run_bass_kernel_spmd(nc, in_maps, core_ids=list(range(NCORE)))
    LAST_RESULT = res
    return _unshard(res.results)
